# revision 1
# baseline (speedup 1.0000x reference)
"""Trainium2 Bass kernel for a 2-layer GCN + FC head (nn_CNNGNNModel).

Reference computation (PyG GCNConv semantics, symmetric normalization with
self-loops):
    deg[i]  = in-degree(i) + 1 ;  dinv = deg^-0.5
    A_hat   = D^-1/2 (A + I) D^-1/2   (aggregation by destination)
    h1 = relu(A_hat @ (x @ W1) + b1)
    h2 = relu(A_hat @ (h1 @ W2) + b2)
    out = h2 @ Wfc + bfc

Key algebraic trick: the per-edge weight dinv[src]*dinv[dst] is separable, so
we scale node features by dinv on the way out of each matmul (source side) and
scale the aggregate by dinv after the segment sum (dest side).  Message
passing then becomes a pure gather + segmented sum.

Distribution (8 NeuronCores, SPMD single program):
  - Nodes are sharded by id range: core c owns dests [c*12500, (c+1)*12500),
    padded to 12544 = 98*128 slots.  Within a core, dests are permuted
    (degree/chunk-count lexsort) to minimize gather padding; all index
    plumbing is precomputed on the host in "position" space
    pos = owner*12544 + slot.
  - Each layer: local matmul of the core's node block -> dinv-scaled bf16
    features -> AllGather to a full table [100352, 256] bf16 -> per-block
    dma_gather of incoming messages -> DVE fold-tree segmented sum -> relu.
  - dma_gather indices are int16 (<= 32767); the table is addressed in 4
    residue "chunks" of 25088 rows via the in_ AP base offset, so each block
    issues up to 4 gather calls (one per chunk) padded per-(block,chunk) to
    the max count over the 128 dest slots of the block (zero-row padding).
  - Weights are replicated; output [12544, 1000] f32 per core is reassembled
    (inverse permutation) on the host.
"""

import numpy as np
import ml_dtypes

import concourse.bass as bass
import concourse.bacc as bacc
import concourse.mybir as mybir
import concourse.tile as tile
from concourse.bass_utils import run_bass_kernel_spmd
from concourse.masks import make_identity

BF16 = ml_dtypes.bfloat16

N_CORES = 8
N_NODES = 100000
IN_DIM = 512
HID = 256
NCLS = 1000
NLOC = 12500          # real dests per core
SLOTS = 12544         # padded dests per core (98 blocks of 128)
BLOCKS = SLOTS // 128  # 98
NCHUNK = 4
CH_ROWS = 2 * SLOTS   # 25088 rows per chunk (= 2 cores)
ZERO_LOCAL = 12543    # chunk-local row guaranteed to be a zero pad row
LCAP = 56             # max gather-tile columns per round
P = 128


def _wrap_idx(flat_idx: np.ndarray) -> np.ndarray:
    """Wrap a flat int16 index array [n] (n % 16 == 0) into the dma_gather
    SBUF layout [128, n//16]: position j -> (partition j%16, column j//16),
    replicated across the eight 16-partition bands."""
    n = flat_idx.shape[0]
    band = flat_idx.reshape(n // 16, 16).T  # [16, n//16]
    return np.tile(band, (8, 1)).astype(np.int16)


def _preprocess(x, edge_index, W1, b1, W2, b2, Wfc, bfc):
    """All host-side graph preprocessing. Returns (plan, in_maps, ids_order)."""
    row = np.asarray(edge_index[0], dtype=np.int64)
    col = np.asarray(edge_index[1], dtype=np.int64)

    deg = np.bincount(col, minlength=N_NODES).astype(np.int64) + 1
    dinv = (1.0 / np.sqrt(deg.astype(np.float32))).astype(np.float32)

    # --- node -> (core, slot) assignment, built to minimize gather padding.
    # A node's "chunk" as a message SOURCE is core//2 (4 chunks of 2 cores,
    # 25088 table rows each; dma_gather int16 indices only reach 32767 rows,
    # hence the chunked gather).  We greedily color nodes into the 4 chunks
    # so that every dest's in-edges are spread evenly over chunks; then the
    # per-(block,chunk) max padding is small.  Within a chunk, dests are
    # sorted by their count vector and striped across the chunk's 2 cores.
    rng = np.random.default_rng(12345)
    all_src0 = np.concatenate([row, np.arange(N_NODES)])
    all_dst0 = np.concatenate([col, np.arange(N_NODES)])
    o = np.argsort(all_src0, kind="stable")
    sr = all_src0[o]
    sc = all_dst0[o]
    starts = np.searchsorted(sr, np.arange(N_NODES + 1))
    deg_out = np.diff(starts)
    target = deg.astype(np.float32) / NCHUNK

    CAP = CH_ROWS - 64  # leave pad rows in every chunk
    color = np.full(N_NODES, -1, np.int8)
    kmat = np.zeros((N_NODES, NCHUNK), np.int32)
    sizes = np.zeros(NCHUNK, np.int64)
    order_src = rng.permutation(N_NODES)
    B = 1000
    for i in range(0, N_NODES, B):
        batch = order_src[i:i + B]
        reps = deg_out[batch]
        idx = np.concatenate(
            [np.arange(starts[s], starts[s + 1]) for s in batch]
        )
        dsts = sc[idx]
        srcrep = np.repeat(np.arange(len(batch)), reps)
        dev = kmat[dsts].astype(np.float32) - target[dsts][:, None]
        score = np.zeros((len(batch), NCHUNK), np.float32)
        np.add.at(score, srcrep, dev)
        score += (sizes / CAP).astype(np.float32) * 0.5 * reps[:, None]
        score[:, sizes >= CAP] = 1e18
        ch = score.argmin(1).astype(np.int8)
        color[batch] = ch
        np.add.at(sizes, ch, 1)
        np.add.at(kmat, (dsts, ch[srcrep]), 1)

    slot = np.empty(N_NODES, np.int64)
    core_of = np.empty(N_NODES, np.int64)
    ids_order = []
    n_core = [0] * N_CORES
    for q in range(NCHUNK):
        nodes_q = np.where(color == q)[0]
        kk = kmat[nodes_q]
        mm = kk.max(1)
        o2 = np.lexsort((-kk[:, 3], -kk[:, 2], -kk[:, 1], -kk[:, 0], -mm))
        nq = nodes_q[o2]
        r = np.arange(len(nq))
        core_of[nq] = 2 * q + (r % 2)
        slot[nq] = (r // 256) * P + (r % 256) // 2
    pos = core_of * SLOTS + slot
    for c in range(N_CORES):
        ids = np.where(core_of == c)[0]
        ids = ids[np.argsort(slot[ids])]
        ids_order.append(ids)
        n_core[c] = len(ids)
        assert np.array_equal(slot[ids], np.arange(len(ids)))

    # --- edge lists sorted by (dest position, src chunk)
    # include self loops as edges
    all_src = np.concatenate([row, np.arange(N_NODES)])
    all_dst = np.concatenate([col, np.arange(N_NODES)])
    dst_pos = pos[all_dst]
    src_pos = pos[all_src]
    s_chunk = src_pos // CH_ROWS
    order = np.lexsort((s_chunk, dst_pos))
    dst_pos = dst_pos[order]
    src_pos = src_pos[order]
    s_chunk = s_chunk[order]

    # per (dest position, chunk) counts and CSR starts
    key = dst_pos * NCHUNK + s_chunk
    kcnt = np.bincount(key, minlength=N_CORES * SLOTS * NCHUNK).reshape(
        N_CORES, SLOTS, NCHUNK
    )
    csr = np.zeros(N_CORES * SLOTS * NCHUNK + 1, np.int64)
    np.cumsum(kcnt.ravel(), out=csr[1:])

    # per-(block, chunk) K shared across all cores
    kblk = kcnt.reshape(N_CORES, BLOCKS, P, NCHUNK)
    Kbq = kblk.max(axis=(0, 2))  # [BLOCKS, NCHUNK]

    real_edges = int(kcnt.sum())
    padded_edges = int(Kbq.sum() * P * N_CORES)
    plan_inflation = padded_edges / real_edges

    # round packing: per block, greedily pack chunks into rounds of <= LCAP cols
    rounds = []  # list per block: list of rounds, each = list of (q, Kq, Cq_in_round)
    for b in range(BLOCKS):
        rs = []
        cur = []
        cur_cols = 0
        for q in range(NCHUNK):
            kq = int(Kbq[b, q])
            if kq == 0:
                continue
            take = 0
            while take < kq:
                room = LCAP - cur_cols
                if room == 0:
                    rs.append(cur)
                    cur = []
                    cur_cols = 0
                    room = LCAP
                # SWDGE descriptor-ring limit: dma_gather crashes above
                # ~1024 indices per call -> cap each call at 8 columns.
                part = min(room, kq - take, 8)
                cur.append((q, take, part, cur_cols))
                cur_cols += part
                take += part
        if cur:
            rs.append(cur)
        rounds.append(rs)

    # --- build per-core index arrays (shared shapes; values differ)
    # also build layer-agnostic gather call plan with compile-time offsets
    idx_arrays = [[] for _ in range(N_CORES)]
    call_plan = []  # per block: list of rounds: list of (q, ncols, round_col, idx_off16)
    off16 = 0
    p_ar = np.arange(P)
    for b in range(BLOCKS):
        blk_plan = []
        for rs in rounds[b]:
            r_plan = []
            for (q, take0, ncols, rcol) in rs:
                n_idx = P * ncols
                r_plan.append((q, ncols, rcol, off16))
                off16 += P * (n_idx // 16)  # wrapped int16 elements: 128 * n/16
                for c in range(N_CORES):
                    slots_g = c * SLOTS + b * P + p_ar  # global dest rows
                    base = csr[(slots_g * NCHUNK + q)]
                    kreal = kcnt[c, b * P + p_ar, q]
                    # position j = col*128 + p ; col in [take0, take0+ncols)
                    jj = take0 + np.arange(ncols)
                    # idx [ncols, P]
                    gather_rows = np.full((ncols, P), q * CH_ROWS + (q * 2) * SLOTS,
                                          np.int64)
                    valid = jj[:, None] < kreal[None, :]
                    src_take = np.minimum(jj[:, None], kreal[None, :] - 1)
                    rowsel = src_pos[base[None, :] + src_take]
                    zero_row = q * CH_ROWS + ZERO_LOCAL  # global pos of a 0 row
                    gather_rows = np.where(valid, rowsel, zero_row)
                    local = (gather_rows - q * CH_ROWS).astype(np.int16)
                    idx_arrays[c].append(_wrap_idx(local.reshape(-1)))
            blk_plan.append(r_plan)
        call_plan.append(blk_plan)

    idx_in = [np.concatenate([a.reshape(-1) for a in idx_arrays[c]])
              for c in range(N_CORES)]
    assert idx_in[0].shape[0] == off16

    # --- per-core dense inputs
    xb = np.ascontiguousarray(x).astype(BF16)
    in_maps = []
    w1_in = np.ascontiguousarray(
        W1.astype(BF16).reshape(NCHUNK, P, HID).transpose(1, 0, 2).reshape(P, NCHUNK * HID)
    )
    w2_in = np.ascontiguousarray(
        W2.astype(BF16).reshape(2, P, HID).transpose(1, 0, 2).reshape(P, 2 * HID)
    )
    wfc_in = np.ascontiguousarray(
        Wfc.astype(BF16).reshape(2, P, NCLS).transpose(1, 0, 2).reshape(P, 2 * NCLS)
    )
    has_b1 = bool(np.any(b1)) ; has_b2 = bool(np.any(b2)) ; has_bfc = bool(np.any(bfc))
    b1_in = np.tile(np.asarray(b1, np.float32)[None, :], (P, 1))
    b2_in = np.tile(np.asarray(b2, np.float32)[None, :], (P, 1))
    bfc_in = np.tile(np.asarray(bfc, np.float32)[None, :], (P, 1))

    for c in range(N_CORES):
        A = np.zeros((SLOTS, IN_DIM), BF16)
        A[:n_core[c]] = xb[ids_order[c]]
        xtt = np.ascontiguousarray(
            A.reshape(BLOCKS, P, NCHUNK, P).transpose(0, 3, 2, 1).reshape(BLOCKS, P, IN_DIM)
        )
        dv = np.ones(SLOTS, np.float32)
        dv[:n_core[c]] = dinv[ids_order[c]]
        dvp = np.ascontiguousarray(dv.reshape(BLOCKS, P).T)  # [128, 98]
        m = {
            "xtt": xtt,
            "dinvp": dvp,
            "idxs": idx_in[c],
            "w1": w1_in,
            "w2": w2_in,
            "wfc": wfc_in,
        }
        if has_b1:
            m["b1b"] = b1_in
        if has_b2:
            m["b2b"] = b2_in
        if has_bfc:
            m["bfcb"] = bfc_in
        in_maps.append(m)

    plan = {
        "call_plan": call_plan,
        "rounds_cols": [[sum(p[1] for p in r) for r in blk] for blk in call_plan],
        "idx_total": off16,
        "has_b1": has_b1,
        "has_b2": has_b2,
        "has_bfc": has_bfc,
        "inflation": plan_inflation,
        "n_core": n_core,
    }
    return plan, in_maps, ids_order


def _build_program(plan, sim_single_core=False, stop_after="full"):
    """Build the SPMD Bass program (one program, all cores).

    stop_after: one of "mm1", "ag1", "g1", "mm2", "ag2", "g2", "full" —
    truncates the program after that phase (for bisection/debug)."""
    STAGES = ["mm1", "ag1", "g1a", "g1b", "g1c", "g1", "mm2", "ag2", "g2", "full"]
    stop_idx = STAGES.index(stop_after)
    detail = {"g1a": 1, "g1b": 2, "g1c": 3}.get(stop_after, 4)
    nc = bacc.Bacc("TRN2", target_bir_lowering=False, debug=False,
                   num_devices=N_CORES)
    dt = mybir.dt

    xtt = nc.dram_tensor("xtt", [BLOCKS, P, IN_DIM], dt.bfloat16, kind="ExternalInput")
    dinvp = nc.dram_tensor("dinvp", [P, BLOCKS], dt.float32, kind="ExternalInput")
    idxs = nc.dram_tensor("idxs", [plan["idx_total"]], dt.int16, kind="ExternalInput")
    w1 = nc.dram_tensor("w1", [P, NCHUNK * HID], dt.bfloat16, kind="ExternalInput")
    w2 = nc.dram_tensor("w2", [P, 2 * HID], dt.bfloat16, kind="ExternalInput")
    wfc = nc.dram_tensor("wfc", [P, 2 * NCLS], dt.bfloat16, kind="ExternalInput")
    b1b = (nc.dram_tensor("b1b", [P, HID], dt.float32, kind="ExternalInput")
           if plan["has_b1"] else None)
    b2b = (nc.dram_tensor("b2b", [P, HID], dt.float32, kind="ExternalInput")
           if plan["has_b2"] else None)
    bfcb = (nc.dram_tensor("bfcb", [P, NCLS], dt.float32, kind="ExternalInput")
            if plan["has_bfc"] else None)
    out = nc.dram_tensor("out", [SLOTS, NCLS], dt.float32, kind="ExternalOutput")

    hloc1 = nc.dram_tensor("hloc1", [SLOTS, HID], dt.bfloat16)
    hloc2 = nc.dram_tensor("hloc2", [SLOTS, HID], dt.bfloat16)
    hfull1 = nc.dram_tensor("hfull1", [N_CORES * SLOTS, HID], dt.bfloat16,
                            addr_space="Shared")
    hfull2 = nc.dram_tensor("hfull2", [N_CORES * SLOTS, HID], dt.bfloat16,
                            addr_space="Shared")
    h1T = nc.dram_tensor("h1T", [BLOCKS, P, HID], dt.bfloat16)
    h2T = nc.dram_tensor("h2T", [BLOCKS, P, HID], dt.bfloat16)

    call_plan = plan["call_plan"]
    rounds_cols = plan["rounds_cols"]

    with tile.TileContext(nc) as tc:
        with (
            tc.tile_pool(name="const", bufs=1) as constp,
            tc.tile_pool(name="xt", bufs=3) as xtp,
            tc.tile_pool(name="hl", bufs=3) as hlp,
            tc.tile_pool(name="idx", bufs=6) as idxp,
            tc.tile_pool(name="g", bufs=3) as gp,
            tc.tile_pool(name="hsmall", bufs=4) as hsp,
            tc.tile_pool(name="fco", bufs=2) as fcop,
            tc.tile_pool(name="mmps", bufs=2, space="PSUM") as mmps,
            tc.tile_pool(name="tpps", bufs=2, space="PSUM") as tpps,
            tc.tile_pool(name="fcps", bufs=2, space="PSUM") as fcps,
        ):
            # resident constants
            w1_sb = constp.tile([P, NCHUNK * HID], dt.bfloat16)
            nc.sync.dma_start(out=w1_sb[:], in_=w1[:])
            w2_sb = constp.tile([P, 2 * HID], dt.bfloat16)
            nc.sync.dma_start(out=w2_sb[:], in_=w2[:])
            wfc_sb = constp.tile([P, 2 * NCLS], dt.bfloat16)
            nc.sync.dma_start(out=wfc_sb[:], in_=wfc[:])
            dv_sb = constp.tile([P, BLOCKS], dt.float32)
            nc.sync.dma_start(out=dv_sb[:], in_=dinvp[:])
            ident = constp.tile([P, P], dt.bfloat16)
            make_identity(nc, ident[:])
            b1_sb = b2_sb = bfc_sb = None
            if b1b is not None:
                b1_sb = constp.tile([P, HID], dt.float32)
                nc.sync.dma_start(out=b1_sb[:], in_=b1b[:])
            if b2b is not None:
                b2_sb = constp.tile([P, HID], dt.float32)
                nc.sync.dma_start(out=b2_sb[:], in_=b2b[:])
            if bfcb is not None:
                bfc_sb = constp.tile([P, NCLS], dt.float32)
                nc.sync.dma_start(out=bfc_sb[:], in_=bfcb[:])

            def layer_matmul(src_dram, w_sb, nk, hloc):
                """hloc[mb] = dinv * (A @ W) as bf16, A tiles from src_dram."""
                for mb in range(BLOCKS):
                    at = xtp.tile([P, nk * P], dt.bfloat16, tag="xt")
                    nc.sync.dma_start(out=at[:], in_=src_dram[mb])
                    ps = mmps.tile([P, HID], dt.float32, space="PSUM", tag="mm")
                    for k in range(nk):
                        nc.tensor.matmul(
                            out=ps[:],
                            lhsT=at[:, k * P:(k + 1) * P],
                            rhs=w_sb[:, k * HID:(k + 1) * HID],
                            start=(k == 0),
                            stop=(k == nk - 1),
                        )
                    hl = hlp.tile([P, HID], dt.bfloat16, tag="hl")
                    nc.scalar.activation(
                        out=hl[:], in_=ps[:],
                        func=mybir.ActivationFunctionType.Copy,
                        scale=dv_sb[:, mb:mb + 1],
                    )
                    nc.sync.dma_start(out=hloc[mb * P:(mb + 1) * P, :], in_=hl[:])

            def all_gather(hloc, hfull):
                if sim_single_core:
                    nc.sync.dma_start(out=hfull[0:SLOTS, :], in_=hloc[:])
                else:
                    nc.gpsimd.collective_compute(
                        "AllGather",
                        mybir.AluOpType.bypass,
                        replica_groups=[list(range(N_CORES))],
                        ins=[hloc[:]],
                        outs=[hfull[:]],
                    )

            def gather_layer(hfull, b_sb, hT, detail=4):
                """h = relu(dinv * segsum(gather(hfull))) (+bias);
                writes transposed tiles to hT."""
                for b in range(BLOCKS):
                    partials = []
                    for r, r_plan in enumerate(call_plan[b]):
                        ncols = rounds_cols[b][r]
                        g = gp.tile([P, LCAP * HID], dt.bfloat16, tag="g")
                        for (q, kq, rcol, ioff) in r_plan:
                            it = idxp.tile([P, LCAP * 8], dt.int16, tag="idx")
                            n16 = P * kq * 8
                            nc.sync.dma_start(
                                out=it[:, :kq * 8],
                                in_=idxs[ioff:ioff + n16].rearrange(
                                    "(p s) -> p s", p=P),
                            )
                            nidx = P * kq
                            nc.gpsimd.dma_gather(
                                g[:, rcol * HID:(rcol + kq) * HID].rearrange(
                                    "p (l d) -> p l d", d=HID),
                                hfull[q * CH_ROWS:(q + 1) * CH_ROWS, :],
                                it[:, :kq * 8],
                                nidx,
                                nidx,
                                HID,
                            )
                        # fold tree over ncols columns
                        cur = ncols if detail >= 2 else 1
                        while cur > 1:
                            half = cur // 2
                            keep = cur - half
                            nc.vector.tensor_tensor(
                                out=g[:, 0:half * HID],
                                in0=g[:, 0:half * HID],
                                in1=g[:, keep * HID:(keep + half) * HID],
                                op=mybir.AluOpType.add,
                            )
                            cur = keep
                        partials.append(g)
                    for r in range(1, len(partials)):
                        nc.vector.tensor_tensor(
                            out=partials[0][:, 0:HID],
                            in0=partials[0][:, 0:HID],
                            in1=partials[r][:, 0:HID],
                            op=mybir.AluOpType.add,
                        )
                    agg = partials[0]
                    if detail < 3:
                        continue
                    h = hsp.tile([P, HID], dt.bfloat16, tag="h")
                    if b_sb is not None:
                        hf = hsp.tile([P, HID], dt.float32, tag="hf")
                        nc.vector.tensor_scalar(
                            out=hf[:], in0=agg[:, 0:HID],
                            scalar1=dv_sb[:, b:b + 1], scalar2=None,
                            op0=mybir.AluOpType.mult,
                        )
                        nc.vector.tensor_tensor(
                            out=hf[:], in0=hf[:], in1=b_sb[:],
                            op=mybir.AluOpType.add,
                        )
                        nc.scalar.activation(
                            out=h[:], in_=hf[:],
                            func=mybir.ActivationFunctionType.Relu,
                        )
                    else:
                        nc.scalar.activation(
                            out=h[:], in_=agg[:, 0:HID],
                            func=mybir.ActivationFunctionType.Relu,
                            scale=dv_sb[:, b:b + 1],
                        )
                    if detail < 4:
                        continue
                    # transpose h [128 nodes, 256 feat] -> hT[mb] [128 feat-part, 256]
                    ht = hsp.tile([P, HID], dt.bfloat16, tag="ht")
                    for k in range(2):
                        tp = tpps.tile([P, P], dt.bfloat16, space="PSUM", tag="tp")
                        nc.tensor.transpose(
                            out=tp[:], in_=h[:, k * P:(k + 1) * P], identity=ident[:]
                        )
                        nc.scalar.copy(out=ht[:, k * P:(k + 1) * P], in_=tp[:])
                    nc.sync.dma_start(out=hT[b], in_=ht[:])

            # ---- layer 1
            layer_matmul(xtt, w1_sb, NCHUNK, hloc1)
            if stop_idx >= 1:
                all_gather(hloc1, hfull1)
            if stop_idx >= 2:
                gather_layer(hfull1, b1_sb, h1T, detail=detail)
            # ---- layer 2
            if stop_idx >= 6:
                layer_matmul(h1T, w2_sb, 2, hloc2)
            if stop_idx >= 7:
                all_gather(hloc2, hfull2)
            if stop_idx >= 8:
                gather_layer(hfull2, b2_sb, h2T)
            # ---- FC head
            for mb in range(BLOCKS if stop_idx >= 9 else 0):
                at = xtp.tile([P, 2 * P], dt.bfloat16, tag="xt")
                nc.sync.dma_start(out=at[:], in_=h2T[mb])
                fo = fcop.tile([P, NCLS], dt.float32, tag="fco")
                for n in range(2):
                    ps = fcps.tile([P, NCLS // 2], dt.float32, space="PSUM", tag="fc")
                    for k in range(2):
                        nc.tensor.matmul(
                            out=ps[:],
                            lhsT=at[:, k * P:(k + 1) * P],
                            rhs=wfc_sb[:, k * NCLS + n * (NCLS // 2):
                                       k * NCLS + (n + 1) * (NCLS // 2)],
                            start=(k == 0),
                            stop=(k == 1),
                        )
                        pass
                    nc.vector.tensor_copy(
                        out=fo[:, n * (NCLS // 2):(n + 1) * (NCLS // 2)], in_=ps[:]
                    )
                if bfc_sb is not None:
                    nc.vector.tensor_tensor(
                        out=fo[:], in0=fo[:], in1=bfc_sb[:],
                        op=mybir.AluOpType.add,
                    )
                nc.sync.dma_start(out=out[mb * P:(mb + 1) * P, :], in_=fo[:])

    nc.compile()
    return nc


_CACHE = {}


def kernel(x, edge_index, W1, b1, W2, b2, Wfc, bfc):
    x = np.asarray(x)
    plan, in_maps, ids_order = _preprocess(x, edge_index, W1, b1, W2, b2, Wfc, bfc)
    nc = _build_program(plan)
    res = run_bass_kernel_spmd(nc, in_maps, core_ids=list(range(N_CORES)))
    full = np.empty((N_NODES, NCLS), np.float32)
    for c in range(N_CORES):
        full[ids_order[c]] = res.results[c]["out"][: len(ids_order[c])]
    return full



# revision 3
# speedup vs baseline: 1.3614x; 1.3614x over previous
"""Trainium2 Bass kernel for a 2-layer GCN + FC head (nn_CNNGNNModel).

Reference computation (PyG GCNConv semantics, symmetric normalization with
self-loops):
    deg[i]  = in-degree(i) + 1 ;  dinv = deg^-0.5
    A_hat   = D^-1/2 (A + I) D^-1/2   (aggregation by destination)
    h1 = relu(A_hat @ (x @ W1) + b1)
    h2 = relu(A_hat @ (h1 @ W2) + b2)
    out = h2 @ Wfc + bfc

The per-edge weight dinv[src]*dinv[dst] is separable: source-side dinv is
folded into the feature rows on the way out of each matmul; dest-side dinv is
folded into the one-hot aggregation matrices.

Distribution (8 NeuronCores, SPMD single program):
  - Nodes are assigned host-side to table positions pos in [0, 100352); core
    c owns positions [c*12544, (c+1)*12544) both as destinations (12544 =
    98 blocks of 128 dest slots) and as its shard of the source-feature
    table.  Positions are chosen to balance per-(block, source-window) edge
    counts (greedy window coloring + degree-snake block packing).
  - Each layer: local matmul of the core's node block -> dinv-scaled bf16
    features -> AllGather to a full table [100352, 256] bf16 -> per dest
    block, gather incoming source rows in rounds of 128 edges (dma_gather,
    <=1024 indices per call; the int16 index limit splits the table into 5
    windows of <=20096 rows addressed via the in_ AP base offset).
  - Aggregation is done on the Tensor engine: per round, a [128 edge x 128
    dest] one-hot matrix M (M[e, d] = dinv_dst[e] if dest(e)==d) is built on
    DVE via tensor_scalar(is_equal, mult) against an iota constant, then
    aggT[f, d] += G[e, f]^T M accumulates in PSUM, transposed so the result
    feeds the next matmul's lhsT directly (no transposes anywhere).
  - Weights are replicated; output [12544, 1000] bf16 per core is
    reassembled (inverse position map) and upcast to f32 on the host.
"""

import numpy as np
import ml_dtypes

import concourse.bass as bass
import concourse.bacc as bacc
import concourse.mybir as mybir
import concourse.tile as tile
from concourse.bass_utils import run_bass_kernel_spmd

BF16 = ml_dtypes.bfloat16

N_CORES = 8
N_NODES = 100000
IN_DIM = 512
HID = 256
NCLS = 1000
P = 128
SLOTS = 12544            # positions per core (98 blocks of 128)
BLOCKS = SLOTS // P      # 98
NPOS = N_CORES * SLOTS   # 100352 table rows
NW = 5                   # gather windows (int16 index limit)
W_BLOCKS = [157, 157, 157, 157, 156]          # blocks per window
WB = np.concatenate([[0], np.cumsum(W_BLOCKS)])  # window block boundaries
W_ROWS0 = WB * P                              # window row bases
MAX_CALL_ROUNDS = 8      # 8*128 = 1024 idx per dma_gather call (HW limit)


def _wrap_idx(flat_idx: np.ndarray) -> np.ndarray:
    """Wrap a flat int16 index array [n] (n % 16 == 0) into the dma_gather
    SBUF layout [128, n//16]: position j -> (partition j%16, column j//16),
    replicated across the eight 16-partition bands."""
    n = flat_idx.shape[0]
    band = flat_idx.reshape(n // 16, 16).T  # [16, n//16]
    return np.tile(band, (8, 1)).astype(np.int16)


def _assign_positions(row, col, deg, rng):
    """Color nodes into the 5 source windows (balancing each destination's
    in-edge window spread), then snake-pack each window's nodes into its
    dest blocks by in-degree profile.  Returns pos[node]."""
    w_slots = np.array([b * P for b in W_BLOCKS])
    caps = w_slots - np.array([70, 70, 70, 70, 72])
    frac = w_slots / float(NPOS)

    # out-edge CSR (self-loops included: node i has an out-edge to itself)
    all_src = np.concatenate([row, np.arange(N_NODES)])
    all_dst = np.concatenate([col, np.arange(N_NODES)])
    o = np.argsort(all_src, kind="stable")
    sr = all_src[o]
    sc = all_dst[o]
    starts = np.searchsorted(sr, np.arange(N_NODES + 1))
    deg_out = np.diff(starts)
    target = deg.astype(np.float32)[:, None] * frac[None, :].astype(np.float32)

    color = np.full(N_NODES, -1, np.int8)
    kmat = np.zeros((N_NODES, NW), np.int32)
    sizes = np.zeros(NW, np.int64)
    order_src = rng.permutation(N_NODES)
    B = 1000
    for i in range(0, N_NODES, B):
        batch = order_src[i:i + B]
        reps = deg_out[batch]
        idx = np.concatenate(
            [np.arange(starts[s], starts[s + 1]) for s in batch]
        )
        dsts = sc[idx]
        srcrep = np.repeat(np.arange(len(batch)), reps)
        dev = kmat[dsts].astype(np.float32) - target[dsts]
        score = np.zeros((len(batch), NW), np.float32)
        np.add.at(score, srcrep, dev)
        score += (sizes / caps).astype(np.float32) * 0.5 * reps[:, None]
        score[:, sizes >= caps] = 1e18
        ch = score.argmin(1).astype(np.int8)
        color[batch] = ch
        np.add.at(sizes, ch, 1)
        np.add.at(kmat, (dsts, ch[srcrep]), 1)

    pos = np.empty(N_NODES, np.int64)
    for w in range(NW):
        nodes_w = np.where(color == w)[0]
        kk = kmat[nodes_w]
        keys = tuple(-kk[:, j] for j in range(NW - 1, -1, -1)) + (-kk.max(1),)
        nodes_w = nodes_w[np.lexsort(keys)]
        nb = W_BLOCKS[w]
        r = np.arange(len(nodes_w))
        p_ = r // nb
        c_ = r % nb
        blk = np.where(p_ % 2 == 0, c_, nb - 1 - c_)
        pos[nodes_w] = (WB[w] + blk) * P + p_
    return pos


def _preprocess(x, edge_index, W1, b1, W2, b2, Wfc, bfc):
    """Host-side graph preprocessing. Returns (plan, in_maps, pos)."""
    row = np.asarray(edge_index[0], dtype=np.int64)
    col = np.asarray(edge_index[1], dtype=np.int64)

    deg = np.bincount(col, minlength=N_NODES).astype(np.int64) + 1
    dinv = (1.0 / np.sqrt(deg.astype(np.float32))).astype(np.float32)

    rng = np.random.default_rng(12345)
    pos = _assign_positions(row, col, deg, rng)

    # zero (pad) rows per window: highest snake rank of each window is free
    occupied = np.zeros(NPOS, bool)
    occupied[pos] = True
    zrow_local = np.empty(NW, np.int64)
    for w in range(NW):
        free = np.where(~occupied[W_ROWS0[w]:W_ROWS0[w + 1]])[0]
        assert len(free) > 0
        zrow_local[w] = free[-1]

    # --- edge lists (self-loops included) in position space
    all_src = np.concatenate([row, np.arange(N_NODES)])
    all_dst = np.concatenate([col, np.arange(N_NODES)])
    spos = pos[all_src]
    dpos = pos[all_dst]
    w_e = np.searchsorted(W_ROWS0[1:], spos, side="right")
    lidx = (spos - W_ROWS0[w_e]).astype(np.int64)
    gb = dpos // P
    slot = (dpos % P).astype(np.float32)
    core_e = gb // BLOCKS
    lb_e = gb % BLOCKS
    dinv_e = dinv[all_dst]

    # counts per (core, lb, w) and the shared round plan
    key_full = (core_e * BLOCKS + lb_e) * NW + w_e
    cnt = np.bincount(key_full, minlength=N_CORES * BLOCKS * NW).reshape(
        N_CORES, BLOCKS, NW
    )
    Rbw = -(-cnt.max(axis=0) // P)  # [BLOCKS, NW] rounds (ceil)
    R_total = int(Rbw.sum())
    real_edges = int(cnt.sum())
    inflation = R_total * P * N_CORES / real_edges

    # call plan: per (lb, w) chunks of <= MAX_CALL_ROUNDS rounds
    # entries: (lb, w, r0_global, rounds, idx8_off)
    rbase = np.zeros((BLOCKS, NW), np.int64)
    np.cumsum(Rbw.ravel()[:-1], out=rbase.ravel()[1:])
    calls = []
    blk_calls = [[] for _ in range(BLOCKS)]
    idx8_off = 0
    blk_idx8 = np.zeros((BLOCKS, 2), np.int64)  # per-block idx8 [start, len]
    for lb in range(BLOCKS):
        blk_idx8[lb, 0] = idx8_off
        for w in range(NW):
            R = int(Rbw[lb, w])
            r0 = int(rbase[lb, w])
            taken = 0
            while taken < R:
                ch = min(MAX_CALL_ROUNDS, R - taken)
                c_rec = (lb, w, r0 + taken, ch, idx8_off)
                calls.append(c_rec)
                blk_calls[lb].append(c_rec)
                idx8_off += ch * 8
                taken += ch
        blk_idx8[lb, 1] = idx8_off - blk_idx8[lb, 0]
    idx8_total = idx8_off

    # --- per-core gather metadata
    order = np.lexsort((w_e, lb_e, core_e))
    so_core = core_e[order]
    so_key = (lb_e[order] * NW + w_e[order]).astype(np.int64)
    so_lidx = lidx[order]
    so_slot = slot[order]
    so_dinv = dinv_e[order]
    core_starts = np.searchsorted(so_core, np.arange(N_CORES + 1))

    rbase_flat = rbase.ravel()
    Rbw_flat = Rbw.ravel()
    in_maps = []
    xb = np.ascontiguousarray(x).astype(BF16)
    w1_in = np.ascontiguousarray(
        np.asarray(W1).astype(BF16).reshape(4, P, HID).transpose(1, 0, 2).reshape(P, 4 * HID)
    )
    w2_in = np.ascontiguousarray(
        np.asarray(W2).astype(BF16).reshape(2, P, HID).transpose(1, 0, 2).reshape(P, 2 * HID)
    )
    wfc_in = np.ascontiguousarray(
        np.asarray(Wfc).astype(BF16).reshape(2, P, NCLS).transpose(1, 0, 2).reshape(P, 2 * NCLS)
    )
    iota_in = np.tile(np.arange(P, dtype=np.float32).astype(BF16)[None, :], (P, 1))
    has_b1 = bool(np.any(b1)); has_b2 = bool(np.any(b2)); has_bfc = bool(np.any(bfc))
    b1_in = np.asarray(b1, np.float32).reshape(2, P).T.copy()   # [128, 2]
    b2_in = np.asarray(b2, np.float32).reshape(2, P).T.copy()
    bfc_in = np.tile(np.asarray(bfc, np.float32).astype(BF16)[None, :], (P, 1))

    group_zrow = np.repeat(zrow_local[None, :], BLOCKS, axis=0).ravel()

    for c in range(N_CORES):
        s0, s1 = core_starts[c], core_starts[c + 1]
        keys_c = so_key[s0:s1]
        # j-within-group for each edge (groups are contiguous, sorted)
        grp_start = np.searchsorted(keys_c, np.arange(BLOCKS * NW))
        jw = np.arange(s1 - s0) - grp_start[keys_c]
        ecol = rbase_flat[keys_c] + jw // P
        erow = jw % P

        destid = np.zeros((P, R_total), np.float32)
        dinvdst = np.ones((P, R_total), np.float32)
        destid[erow, ecol] = so_slot[s0:s1]
        dinvdst[erow, ecol] = so_dinv[s0:s1]

        # flat (round-major) local indices, defaulting to the window's 0-row
        flat_idx = np.repeat(group_zrow, Rbw_flat * P).astype(np.int64)
        flat_idx[ecol * P + erow] = so_lidx[s0:s1]
        assert flat_idx.max() < 32768
        idxs2d = np.empty((P, idx8_total), np.int16)
        for (lb, w, r0, ch, i8) in calls:
            seg = flat_idx[r0 * P:(r0 + ch) * P].astype(np.int16)
            idxs2d[:, i8:i8 + ch * 8] = _wrap_idx(seg)

        # dense per-core inputs
        ids_c = np.where((pos >= c * SLOTS) & (pos < (c + 1) * SLOTS))[0]
        lpos = pos[ids_c] - c * SLOTS
        A = np.zeros((SLOTS, IN_DIM), BF16)
        A[lpos] = xb[ids_c]
        xtt = np.ascontiguousarray(
            A.reshape(BLOCKS, P, 4, P).transpose(0, 3, 2, 1).reshape(BLOCKS, P, IN_DIM)
        )
        dv = np.ones(SLOTS, np.float32)
        dv[lpos] = dinv[ids_c]
        dvp = np.ascontiguousarray(dv.reshape(BLOCKS, P).T)  # [128, 98]

        m = {
            "xtt": xtt,
            "dinvp": dvp,
            "idxs": idxs2d,
            "destid": destid,
            "dinvdst": dinvdst,
            "iota": iota_in,
            "w1": w1_in,
            "w2": w2_in,
            "wfc": wfc_in,
        }
        if has_b1:
            m["b1h"] = b1_in
        if has_b2:
            m["b2h"] = b2_in
        if has_bfc:
            m["bfcb"] = bfc_in
        in_maps.append(m)

    plan = {
        "blk_calls": blk_calls,
        "blk_idx8": blk_idx8.tolist(),
        "rbase": rbase.tolist(),
        "Rbw": Rbw.tolist(),
        "R_total": R_total,
        "idx8_total": idx8_total,
        "n_calls": len(calls),
        "has_b1": has_b1,
        "has_b2": has_b2,
        "has_bfc": has_bfc,
        "inflation": inflation,
    }
    return plan, in_maps, pos


def _build_program(plan, sim_single_core=False, stop_after="full"):
    """Build the SPMD Bass program (one program, all cores).

    stop_after: one of "mm1", "ag1", "g1", "ag2", "g2", "full" — truncates
    the program after that phase (for bisection/debug)."""
    STAGES = ["mm1", "ag1", "g1", "ag2", "g2", "full"]
    stop_idx = STAGES.index(stop_after)
    nc = bacc.Bacc("TRN2", target_bir_lowering=False, debug=False,
                   num_devices=N_CORES)
    dt = mybir.dt

    R_total = plan["R_total"]
    idx8_total = plan["idx8_total"]
    blk_calls = plan["blk_calls"]
    blk_idx8 = plan["blk_idx8"]
    rbase = plan["rbase"]
    Rbw = plan["Rbw"]

    xtt = nc.dram_tensor("xtt", [BLOCKS, P, IN_DIM], dt.bfloat16, kind="ExternalInput")
    dinvp = nc.dram_tensor("dinvp", [P, BLOCKS], dt.float32, kind="ExternalInput")
    idxs = nc.dram_tensor("idxs", [P, idx8_total], dt.int16, kind="ExternalInput")
    destid = nc.dram_tensor("destid", [P, R_total], dt.float32, kind="ExternalInput")
    dinvdst = nc.dram_tensor("dinvdst", [P, R_total], dt.float32, kind="ExternalInput")
    iota = nc.dram_tensor("iota", [P, P], dt.bfloat16, kind="ExternalInput")
    w1 = nc.dram_tensor("w1", [P, 4 * HID], dt.bfloat16, kind="ExternalInput")
    w2 = nc.dram_tensor("w2", [P, 2 * HID], dt.bfloat16, kind="ExternalInput")
    wfc = nc.dram_tensor("wfc", [P, 2 * NCLS], dt.bfloat16, kind="ExternalInput")
    b1h = (nc.dram_tensor("b1h", [P, 2], dt.float32, kind="ExternalInput")
           if plan["has_b1"] else None)
    b2h = (nc.dram_tensor("b2h", [P, 2], dt.float32, kind="ExternalInput")
           if plan["has_b2"] else None)
    bfcb = (nc.dram_tensor("bfcb", [P, NCLS], dt.bfloat16, kind="ExternalInput")
            if plan["has_bfc"] else None)
    out = nc.dram_tensor("out", [SLOTS, NCLS], dt.bfloat16, kind="ExternalOutput")

    hloc1 = nc.dram_tensor("hloc1", [SLOTS, HID], dt.bfloat16)
    hloc2 = nc.dram_tensor("hloc2", [SLOTS, HID], dt.bfloat16)
    hfull1 = nc.dram_tensor("hfull1", [NPOS, HID], dt.bfloat16, addr_space="Shared")
    hfull2 = nc.dram_tensor("hfull2", [NPOS, HID], dt.bfloat16, addr_space="Shared")

    RELU = mybir.ActivationFunctionType.Relu
    COPY = mybir.ActivationFunctionType.Copy

    with tile.TileContext(nc) as tc:
        with (
            tc.tile_pool(name="const", bufs=1) as constp,
            tc.tile_pool(name="xt", bufs=3) as xtp,
            tc.tile_pool(name="hl", bufs=4) as hlp,
            tc.tile_pool(name="idx", bufs=3) as idxp,
            tc.tile_pool(name="g", bufs=10) as gp,
            tc.tile_pool(name="m", bufs=8) as mp,
            tc.tile_pool(name="at", bufs=4) as atp,
            tc.tile_pool(name="fco", bufs=2) as fcop,
            tc.tile_pool(name="mmps", bufs=2, space="PSUM") as mmps,
            tc.tile_pool(name="aggps", bufs=2, space="PSUM") as aggps,
            tc.tile_pool(name="fcps", bufs=1, space="PSUM") as fcps,
        ):
            # resident constants
            w1_sb = constp.tile([P, 4 * HID], dt.bfloat16)
            nc.sync.dma_start(out=w1_sb[:], in_=w1[:])
            w2_sb = constp.tile([P, 2 * HID], dt.bfloat16)
            nc.sync.dma_start(out=w2_sb[:], in_=w2[:])
            wfc_sb = constp.tile([P, 2 * NCLS], dt.bfloat16)
            nc.sync.dma_start(out=wfc_sb[:], in_=wfc[:])
            dv_sb = constp.tile([P, BLOCKS], dt.float32)
            nc.sync.dma_start(out=dv_sb[:], in_=dinvp[:])
            iota_sb = constp.tile([P, P], dt.bfloat16)
            nc.sync.dma_start(out=iota_sb[:], in_=iota[:])
            dst_sb = constp.tile([P, R_total], dt.float32)
            nc.sync.dma_start(out=dst_sb[:], in_=destid[:])
            dnv_sb = constp.tile([P, R_total], dt.float32)
            nc.sync.dma_start(out=dnv_sb[:], in_=dinvdst[:])
            b1_sb = b2_sb = bfc_sb = None
            if b1h is not None:
                b1_sb = constp.tile([P, 2], dt.float32)
                nc.sync.dma_start(out=b1_sb[:], in_=b1h[:])
            if b2h is not None:
                b2_sb = constp.tile([P, 2], dt.float32)
                nc.sync.dma_start(out=b2_sb[:], in_=b2h[:])
            if bfcb is not None:
                bfc_sb = constp.tile([P, NCLS], dt.bfloat16)
                nc.sync.dma_start(out=bfc_sb[:], in_=bfcb[:])

            def all_gather(hloc, hfull):
                if sim_single_core:
                    nc.sync.dma_start(out=hfull[0:SLOTS, :], in_=hloc[:])
                else:
                    nc.gpsimd.collective_compute(
                        "AllGather",
                        mybir.AluOpType.bypass,
                        replica_groups=[list(range(N_CORES))],
                        ins=[hloc[:]],
                        outs=[hfull[:]],
                    )

            def agg_block(lb, hfull, b_sb):
                """Gather + one-hot matmul aggregation for dest block lb.
                Returns aggT sbuf tile [128, 256] bf16 (partition = feat%128,
                cols 0:128 = feats 0..127, cols 128:256 = feats 128..255,
                free position = dest slot), after relu."""
                i8s, i8n = blk_idx8[lb]
                it = idxp.tile([P, i8n], dt.int16, tag="idx")
                nc.sync.dma_start(out=it[:], in_=idxs[:, i8s:i8s + i8n])
                ps0 = aggps.tile([P, P], dt.float32, space="PSUM", tag="agg0")
                ps1 = aggps.tile([P, P], dt.float32, space="PSUM", tag="agg1")
                n_rounds = sum(Rbw[lb])
                gts = []
                for (lb_, w, r0, ch, i8) in blk_calls[lb]:
                    g = gp.tile([P, MAX_CALL_ROUNDS * HID], dt.bfloat16, tag="g")
                    nidx = ch * P
                    nc.gpsimd.dma_gather(
                        g[:, :ch * HID].rearrange("p (l d) -> p l d", d=HID),
                        hfull[W_ROWS0[w]:W_ROWS0[w + 1], :],
                        it[:, i8 - i8s:i8 - i8s + ch * 8],
                        nidx,
                        nidx,
                        HID,
                    )
                    gts.append((g, r0, ch))
                done = 0
                for (g, r0, ch) in gts:
                    for r in range(ch):
                        rc = r0 + r
                        M = mp.tile([P, P], dt.bfloat16, tag="m")
                        nc.vector.tensor_scalar(
                            out=M[:], in0=iota_sb[:],
                            scalar1=dst_sb[:, rc:rc + 1],
                            scalar2=dnv_sb[:, rc:rc + 1],
                            op0=mybir.AluOpType.is_equal,
                            op1=mybir.AluOpType.mult,
                        )
                        for h in range(2):
                            nc.tensor.matmul(
                                out=(ps0 if h == 0 else ps1)[:],
                                lhsT=g[:, r * HID + h * P: r * HID + h * P + P],
                                rhs=M[:],
                                start=(done == 0),
                                stop=(done == n_rounds - 1),
                            )
                        done += 1
                at = atp.tile([P, HID], dt.bfloat16, tag="at")
                for h in range(2):
                    nc.scalar.activation(
                        out=at[:, h * P:(h + 1) * P],
                        in_=(ps0 if h == 0 else ps1)[:],
                        func=RELU,
                        bias=(b_sb[:, h:h + 1] if b_sb is not None else 0.0),
                    )
                return at

            # ---- layer 1: h1' = dinv * (x @ W1)
            for mb in range(BLOCKS):
                at = xtp.tile([P, IN_DIM], dt.bfloat16, tag="xt")
                nc.sync.dma_start(out=at[:], in_=xtt[mb])
                ps = mmps.tile([P, HID], dt.float32, space="PSUM", tag="mm")
                for k in range(4):
                    nc.tensor.matmul(
                        out=ps[:],
                        lhsT=at[:, k * P:(k + 1) * P],
                        rhs=w1_sb[:, k * HID:(k + 1) * HID],
                        start=(k == 0),
                        stop=(k == 3),
                    )
                hl = hlp.tile([P, HID], dt.bfloat16, tag="hl")
                nc.scalar.activation(
                    out=hl[:], in_=ps[:], func=COPY, scale=dv_sb[:, mb:mb + 1],
                )
                nc.sync.dma_start(out=hloc1[mb * P:(mb + 1) * P, :], in_=hl[:])
            if stop_idx >= 1:
                all_gather(hloc1, hfull1)
            # ---- layer 1 aggregation + layer 2 matmul, fused per block
            for lb in range(BLOCKS if stop_idx >= 2 else 0):
                at1 = agg_block(lb, hfull1, b1_sb)
                ps2 = mmps.tile([P, HID], dt.float32, space="PSUM", tag="mm")
                for k in range(2):
                    nc.tensor.matmul(
                        out=ps2[:],
                        lhsT=at1[:, k * P:(k + 1) * P],
                        rhs=w2_sb[:, k * HID:(k + 1) * HID],
                        start=(k == 0),
                        stop=(k == 1),
                    )
                hl2 = hlp.tile([P, HID], dt.bfloat16, tag="hl")
                nc.scalar.activation(
                    out=hl2[:], in_=ps2[:], func=COPY, scale=dv_sb[:, lb:lb + 1],
                )
                nc.sync.dma_start(out=hloc2[lb * P:(lb + 1) * P, :], in_=hl2[:])
            if stop_idx >= 3:
                all_gather(hloc2, hfull2)
            # ---- layer 2 aggregation + FC head, fused per block
            for lb in range(BLOCKS if stop_idx >= 4 else 0):
                at2 = agg_block(lb, hfull2, b2_sb)
                fo = fcop.tile([P, NCLS], dt.bfloat16, tag="fco")
                for n in range(2):
                    ps = fcps.tile([P, NCLS // 2], dt.float32, space="PSUM",
                                   tag=f"fc{n}")
                    for k in range(2):
                        nc.tensor.matmul(
                            out=ps[:],
                            lhsT=at2[:, k * P:(k + 1) * P],
                            rhs=wfc_sb[:, k * NCLS + n * (NCLS // 2):
                                       k * NCLS + (n + 1) * (NCLS // 2)],
                            start=(k == 0),
                            stop=(k == 1),
                        )
                    nc.scalar.activation(
                        out=fo[:, n * (NCLS // 2):(n + 1) * (NCLS // 2)],
                        in_=ps[:], func=COPY,
                    )
                if bfc_sb is not None:
                    nc.vector.tensor_tensor(
                        out=fo[:], in0=fo[:], in1=bfc_sb[:],
                        op=mybir.AluOpType.add,
                    )
                if stop_idx >= 5:
                    nc.sync.dma_start(out=out[lb * P:(lb + 1) * P, :], in_=fo[:])

    nc.compile()
    return nc


def kernel(x, edge_index, W1, b1, W2, b2, Wfc, bfc):
    x = np.asarray(x)
    plan, in_maps, pos = _preprocess(x, edge_index, W1, b1, W2, b2, Wfc, bfc)
    nc = _build_program(plan)
    res = run_bass_kernel_spmd(nc, in_maps, core_ids=list(range(N_CORES)))
    full = np.empty((N_NODES, NCLS), np.float32)
    core = pos // SLOTS
    lrow = pos % SLOTS
    for c in range(N_CORES):
        sel = core == c
        full[sel] = res.results[c]["out"][lrow[sel]].astype(np.float32)
    return full
